# revision 7
# baseline (speedup 1.0000x reference)
"""Trainium2 Bass kernel for nn_ExtractionLayer.

metric[b,v,f] = sum_p amp[b,f,p] * exp(-c*(vol[v]*filt[f] - q[b,p])^2)
  amp = softmax_p(logits[b,f,p]),  c = 0.5/(sigma+0.001)^2

Sharding: data-parallel over batch B=32 -> 4 b's per core on 8 cores.

Per-core algorithm (2 "sets", each set = 2 b's = 128 (b,p) partition pairs):
  PE pass 1 : S'[(b,p),(f,v)] = x^2 - 2qx via a K=9 bf16 matmul.
              bf16 is 4x faster than fp32 on the PE; fp32-level accuracy
              comes from 3-way hi/mid/lo bf16 splits of x^2, x and -2q.
              Even/odd 512-col chunks use PE row-groups 0/32 and run
              concurrently. The q^2 term folds into the ACT bias.
  ACT pass  : E = exp(-c*S' - c*q^2)  PSUM->SBUF fp16, FD=1536 groups,
              double-buffered PSUM -- this ~66us exp pass is the floor.
  PE pass 2 : per (f, v-half): lhsT = E-slice (128,128) stationary,
              rhs = block-diag softmax weight pair (128,2) moving ->
              small per-group psum tile [128, 24].
  drain     : per group: DVE copy psum->SBUF, DMA -> out (incremental,
              no tail burst); host -> [b,v,f]

All weight packing (X rows = bf16 splits of fil*vol outer product, Wq
rows = splits of -2q, exp bias = -c*q^2) is done on HOST from the tiny
param tensors (vol 256, fil 128, q 256/core) and passed as inputs, so
the device pipeline starts on the first X-piece DMA with no on-device
setup chain.  Softmax stays on device: elt = exp(lt) on ACT, Z via a
PE ones-matmul partition reduction, Wamp = elt * (1/Z) on DVE.
"""

import sys

for _p in ("/opt/trn_rl_repo", "/root/.axon_site/_ro/trn_rl_repo"):
    if _p not in sys.path:
        sys.path.append(_p)

import numpy as np
import ml_dtypes

BF16 = ml_dtypes.bfloat16

B, V, F, P = 32, 256, 128, 64
NCORES = 8
B_LOC = B // NCORES          # 4 batches per core
NSETS = B_LOC // 2           # 2 sets of (2 b's x 64 p) = 128 partitions
NVF = V * F                  # 32768 (f-major: i = f*V + v)
GROUP = 1536                 # ACT free dim (3 PSUM banks); last group ragged
NPIECE = 8                   # X column pieces, 2048 within-band cols each
GROUPS = []
_c0 = 0
while _c0 < NVF:
    GROUPS.append((_c0, min(GROUP, NVF - _c0)))
    _c0 += GROUP

_cache: dict = {}


def _build(minus_c):
    import concourse.tile as tile
    from concourse import bacc, mybir

    fp32 = mybir.dt.float32
    fp16 = mybir.dt.float16
    bf16 = mybir.dt.bfloat16
    AF = mybir.ActivationFunctionType
    OP = mybir.AluOpType
    import concourse.bass as bass

    nc = bacc.Bacc("TRN2", target_bir_lowering=False, debug=False,
                   num_devices=NCORES)

    # host-packed inputs
    d_xp = [nc.dram_tensor(f"xp{p}", [2, 9, 2048], bf16,
                           kind="ExternalInput") for p in range(NPIECE)]
    d_wq = nc.dram_tensor("wq", [NSETS, 9, 128], bf16, kind="ExternalInput")
    d_eb = nc.dram_tensor("eb", [128, NSETS], fp32, kind="ExternalInput")
    d_lt = nc.dram_tensor("lt", [B_LOC * P, F], fp16, kind="ExternalInput")
    d_zb = nc.dram_tensor("zb", [B_LOC * F], fp32)  # Zinv bounce, [b][f]
    # out[s, vp, f, vh, b']; host interleaves
    d_out = nc.dram_tensor("out", [NSETS, 128, F, 2, 2], fp32,
                           kind="ExternalOutput")

    with tile.TileContext(nc) as tc:
        with (
            tc.tile_pool(name="const", bufs=1) as cp,
            tc.tile_pool(name="ering", bufs=4) as ep,
            tc.tile_pool(name="psS", bufs=2, space=bass.MemorySpace.PSUM) as psS,
            tc.tile_pool(name="psG", bufs=2, space=bass.MemorySpace.PSUM) as psG,
        ):
            # ---- queue plan ----
            # scalar : piece 0 at t=0 (before any ACT work), then ACT only
            # sync   : X pieces even, out DMAs (even groups)
            # gpsimd : wq/eb/lt loads, zb/zr chain, X pieces odd,
            #          out DMAs (odd groups)
            Xp = [cp.tile([41, 2048], bf16, tag=f"Xp{p}", name=f"Xp{p}")
                  for p in range(NPIECE)]
            Wq = [cp.tile([41, 128], bf16, tag=f"Wq{s}", name=f"Wq{s}")
                  for s in range(NSETS)]
            eb = cp.tile([128, NSETS], fp32, tag="eb")
            lt_sb = [cp.tile([128, F], fp16, tag=f"lt{s}", name=f"lt{s}")
                     for s in range(NSETS)]

            def piece(p, eng):
                for band in range(2):
                    eng.dma_start(Xp[p][32 * band:32 * band + 9, :],
                                  d_xp[p].ap()[band])

            piece(0, nc.scalar)
            # small, startup-critical loads first on gpsimd
            for s in range(NSETS):
                nc.gpsimd.dma_start(Wq[s][0:9, :], d_wq.ap()[s])
                nc.gpsimd.dma_start(Wq[s][32:41, :], d_wq.ap()[s])
            nc.gpsimd.dma_start(eb[:, :], d_eb.ap())
            for s in range(NSETS):
                nc.gpsimd.dma_start(lt_sb[s][:, :],
                                    d_lt.ap()[s * 128:(s + 1) * 128, :])
            for p in (1, 2, 4, 6):
                piece(p, nc.sync)

            # ---- softmax: elt = exp(lt); Z via PE ones-matmul ----
            elt = []
            for s in range(NSETS):
                e = cp.tile([128, F], fp16, tag=f"elt{s}", name=f"elt{s}")
                nc.scalar.activation(e[:, :], lt_sb[s][:, :], AF.Exp)
                elt.append(e)
            ones2 = cp.tile([128, 2], fp16, tag="ones2")
            nc.vector.memset(ones2[:, :], 0.0)
            nc.vector.memset(ones2[0:64, 0:1], 1.0)
            nc.vector.memset(ones2[64:128, 1:2], 1.0)
            Zc = cp.tile([128, 2 * NSETS], fp32, tag="Zc")
            for s in range(NSETS):
                zp = psG.tile([128, 4 * 6], fp32, tag="G", name=f"zp{s}")
                nc.tensor.matmul(zp[:, 0:2], elt[s][:, :], ones2[:, :],
                                 start=True, stop=True)
                nc.vector.tensor_copy(Zc[:, 2 * s:2 * s + 2], zp[:, 0:2])
            Zinv = cp.tile([128, 2 * NSETS], fp32, tag="Zinv")
            nc.vector.reciprocal(Zinv[:, :], Zc[:, :])
            # bounce to DRAM [b][f] (b = 2s+h), read back partition-bcast
            nc.gpsimd.dma_start(
                bass.AP(tensor=d_zb, offset=0, ap=[[1, 128], [128, 2 * NSETS]]),
                Zinv[:, :])
            zrs = []
            for s in range(NSETS):
                zr = cp.tile([128, F], fp32, tag=f"zr{s}", name=f"zr{s}")
                for h in range(2):
                    nc.gpsimd.dma_start(
                        zr[h * 64:(h + 1) * 64, :],
                        bass.AP(tensor=d_zb, offset=(2 * s + h) * F,
                                ap=[[0, 64], [1, F]]))
                zrs.append(zr)
            for p in (3, 5, 7):
                piece(p, nc.gpsimd)

            # ---- W_amp: block-diag fp16 softmax weights (DVE) ----
            # W_amp[k=(b,p), 2f+h] = amp[b,f,p] for k//64==h else 0
            Wamp = []
            for s in range(NSETS):
                w = cp.tile([128, 2 * F], fp16, tag=f"Wamp{s}", name=f"Wamp{s}")
                nc.vector.memset(w[:, :], 0.0)
                for h in range(2):
                    nc.vector.tensor_tensor(
                        w[h * 64:(h + 1) * 64, h:2 * F:2],
                        elt[s][h * 64:(h + 1) * 64, :],
                        zrs[s][h * 64:(h + 1) * 64, :],
                        OP.mult,
                    )
                Wamp.append(w)

            # ---- main pipeline ----
            oeng = [nc.sync, nc.gpsimd]
            gi = 0
            for s in range(NSETS):
                for (g0, gc) in GROUPS:
                    sS = psS.tile([128, GROUP], fp32, tag="S", name="sS")
                    for ci in range(gc // 512):
                        c = (g0 + ci * 512) // 512        # global chunk
                        band, bloc = c % 2, (c // 2) * 512
                        p, loc = bloc // 2048, bloc % 2048
                        nc.tensor.matmul(
                            sS[:, ci * 512:(ci + 1) * 512],
                            Wq[s][32 * band:32 * band + 9, :],
                            Xp[p][32 * band:32 * band + 9, loc:loc + 512],
                            start=True, stop=True,
                            tile_position=(32 * band, 0),
                        )
                    E = ep.tile([128, GROUP], fp16, tag="E", name="E")
                    nc.scalar.activation(E[:, 0:gc], sS[:, 0:gc], AF.Exp,
                                         scale=float(minus_c),
                                         bias=eb[:, s:s + 1])
                    nf = gc // V
                    sG = psG.tile([128, 4 * 6], fp32, tag="G", name="sG")
                    for fr in range(nf):
                        f = g0 // V + fr                       # global f
                        for vh in range(2):
                            nc.tensor.matmul(
                                sG[:, fr * 4 + vh * 2:fr * 4 + vh * 2 + 2],
                                E[:, fr * V + vh * 128:fr * V + vh * 128 + 128],
                                Wamp[s][:, 2 * f:2 * f + 2],
                                start=True, stop=True,
                            )
                    # drain: psum -> sbuf -> DRAM out[s, vp, f0:f0+nf, vh, b']
                    ob = cp.tile([128, 4 * 6], fp32, tag=f"ob{gi % 4}",
                                 name=f"ob{gi}")
                    nc.vector.tensor_copy(ob[:, 0:4 * nf], sG[:, 0:4 * nf])
                    f0 = g0 // V
                    oeng[gi % 2].dma_start(
                        bass.AP(tensor=d_out,
                                offset=s * (128 * F * 4) + f0 * 4,
                                ap=[[F * 4, 128], [1, 4 * nf]]),
                        ob[:, 0:4 * nf])
                    gi += 1

    nc.compile()
    return nc


def _get_nc(minus_c):
    key = float(minus_c)
    if key not in _cache:
        _cache[key] = _build(key)
    return _cache[key]


def _split3(v):
    h = v.astype(BF16)
    r1 = v - h.astype(np.float32)
    m = r1.astype(BF16)
    r2 = r1 - m.astype(np.float32)
    return h, m, r2.astype(BF16)


def _pack_x(vol, fil):
    """8 pieces [2, 9, 2048] bf16 of packed X rows.
    Within-band chunk j (band b) holds f = 4j + 2b + fi; piece p has
    j in [4p, 4p+4); rows = [ah am al bh bh bh bm bm bl]."""
    x = fil[:, None] * vol[None, :]                     # [F, V]
    ah, am, al = _split3(x * x)
    bh, bm, bl = _split3(x)
    rows = np.stack([ah, am, al, bh, bh, bh, bm, bm, bl])   # [9, F, V]
    pieces = []
    for p in range(NPIECE):
        arr = np.empty((2, 9, 4 * 512), dtype=BF16)
        for b in range(2):
            for jj in range(4):
                for fi in range(2):
                    f = 4 * (4 * p + jj) + 2 * b + fi
                    arr[b, :, jj * 512 + fi * 256:jj * 512 + fi * 256 + 256] \
                        = rows[:, f, :]
        pieces.append(arr)
    return pieces


def kernel(q2_obs_scaled, amplitude_logits, volumes, filters, sigma,
           _trace=False, _tmpdir=None):
    from concourse.bass_utils import run_bass_kernel_spmd

    minus_c = -0.5 / (float(np.asarray(sigma).reshape(())) + 0.001) ** 2
    nc = _get_nc(minus_c)

    q = np.ascontiguousarray(np.asarray(q2_obs_scaled, dtype=np.float32))
    lg = np.asarray(amplitude_logits, dtype=np.float32).reshape(B, F, P)
    vol = np.ascontiguousarray(np.asarray(volumes, dtype=np.float32).reshape(V))
    fil = np.ascontiguousarray(np.asarray(filters, dtype=np.float32).reshape(F))

    xp = _pack_x(vol, fil)                              # shared across cores

    in_maps = []
    for i in range(NCORES):
        bsl = slice(i * B_LOC, (i + 1) * B_LOC)
        qc = q[bsl].reshape(B_LOC * P)
        lgc = lg[bsl]                                    # (B_LOC, F, P)
        wq = np.empty((NSETS, 9, 128), dtype=BF16)
        ebias = np.empty((128, NSETS), dtype=np.float32)
        for s in range(NSETS):
            qs = qc[s * 128:(s + 1) * 128]
            wh, wm, wl = _split3(-2.0 * qs)
            one = np.ones(128, dtype=BF16)
            wq[s] = np.stack([one, one, one, wh, wm, wl, wh, wm, wh])
            ebias[:, s] = minus_c * qs * qs
        m = {f"xp{p}": xp[p] for p in range(NPIECE)}
        m["wq"] = wq
        m["eb"] = ebias
        m["lt"] = np.ascontiguousarray(
            lgc.transpose(0, 2, 1).reshape(B_LOC * P, F)).astype(np.float16)
        in_maps.append(m)

    kw = {}
    if _trace:
        kw = {"trace": True, "tmpdir": _tmpdir}
    res = run_bass_kernel_spmd(nc, in_maps, core_ids=list(range(NCORES)), **kw)

    out = np.empty((B, V, F), dtype=np.float32)
    for i in range(NCORES):
        oc = res.results[i]["out"]                  # (NSETS, 128, F, 2, 2)
        for s in range(NSETS):
            for h in range(2):
                for vh in range(2):
                    out[i * B_LOC + 2 * s + h, vh * 128:(vh + 1) * 128, :] \
                        = oc[s, :, :, vh, h]
    if _trace:
        return out, res
    return out


# revision 13
# speedup vs baseline: 2.6026x; 2.6026x over previous
"""Trainium2 Bass kernel for nn_ExtractionLayer — log-grid table algorithm.

metric[b,v,f] = sum_p amp[b,f,p] * G(x_vf - q_bp),  x_vf = fil_f*vol_v,
G = Gaussian of width sig_eff = sigma+0.001, amp = softmax_p(logits).

Instead of 8.4M exps/core (direct method), build the amp-weighted Gaussian
mixture h_{b,f} on a LOG-SPACED x-grid (NG~1.5K points; exps = NG*256 only)
and evaluate metric by two nested cubic (Catmull-Rom) interpolations:
  position(x) = ln(x/x0)/eps = a_f + c_v   (separable in log space!)
  stage alpha: per-(f,b) window gather of T rows (indirect DMA) + fractional
               defrac with per-partition cubic weights (DVE)
  stage gamma: dense matmul with a host-built interp matrix OC[m, v].

Per-core flow (4 b's/core, data-parallel over batch):
  1. elt=exp(lt) (ACT), block-diag Wamp (DVE), Z via PE ones-matmul (host
     divides output by Z).
  2. K[(bp), g] = exp(-c*(x_g - q)^2) via the 9-row bf16-split matmul + ACT,
     g in NGP cols.  (~1.4K*256 exps = ~3us)
  3. T[rfb, g] = sum_p amp*K: PE matmuls, lhsT = Wamp cols, ACCUMULATED over
     both (bp) set-tiles; rfb = b*128+f on partitions. -> bf16 -> DRAM.
  4. gather: ONE indirect DMA pulls per-(f,b) windows T[rfb, A_f+Mlo-1 : +RUN]
     (coef=1 flat indexing, 512 descriptors).
  5. defrac (stage alpha): T2[rfb, m] = sum_r ca_r(alpha_f)*Tw[rfb, m+r].
  6. PE transpose T2 -> [m-part, rfb], then out[v, rfb] = OC^T @ T2t
     accumulated over m-tiles. -> fp16 -> DRAM out.
Host packs: grid splits, Wq/ebias, OC, defrac coeffs, gather indices.
"""

import sys

for _p in ("/opt/trn_rl_repo", "/root/.axon_site/_ro/trn_rl_repo"):
    if _p not in sys.path:
        sys.path.append(_p)

import numpy as np
import ml_dtypes

BF16 = ml_dtypes.bfloat16

B, V, F, P = 32, 256, 128, 64
NCORES = 8
B_LOC = B // NCORES
NSETS = B_LOC // 2
EPS_FRAC = 0.9

_cache: dict = {}


def _build(minus_c, NGP, MT, M2P, RUNP):
    import concourse.tile as tile
    from concourse import bacc, mybir

    fp32 = mybir.dt.float32
    fp16 = mybir.dt.float16
    bf16 = mybir.dt.bfloat16
    int32 = mybir.dt.int32
    AF = mybir.ActivationFunctionType
    OP = mybir.AluOpType
    import concourse.bass as bass

    NGC = NGP // 512            # K/T column chunks
    MTC = MT // 128             # m partition-tiles

    nc = bacc.Bacc("TRN2", target_bir_lowering=False, debug=False,
                   num_devices=NCORES)

    d_xg = nc.dram_tensor("xg", [9, NGP], bf16, kind="ExternalInput")
    d_wq = nc.dram_tensor("wq", [NSETS, 9, 128], bf16, kind="ExternalInput")
    d_eb = nc.dram_tensor("eb", [128, NSETS], fp32, kind="ExternalInput")
    d_lt = nc.dram_tensor("lt", [B_LOC * P, F], fp16, kind="ExternalInput")
    d_oc = nc.dram_tensor("oc", [MT, 256], bf16, kind="ExternalInput")
    d_ca = nc.dram_tensor("ca", [128, 4], fp32, kind="ExternalInput")
    d_ix = nc.dram_tensor("ix", [128, 4], int32, kind="ExternalInput")
    d_id = nc.dram_tensor("idm", [128, 128], bf16, kind="ExternalInput")
    d_t = nc.dram_tensor("tsc", [4 * 128 * NGP], bf16)     # T scratch, flat
    d_out = nc.dram_tensor("out", [2, 128, 512], fp16, kind="ExternalOutput")
    d_z = nc.dram_tensor("zout", [128, 2 * NSETS], fp32, kind="ExternalOutput")

    with tile.TileContext(nc) as tc:
        with (
            tc.tile_pool(name="const", bufs=1) as cp,
            tc.tile_pool(name="ps", bufs=2, space=bass.MemorySpace.PSUM) as ps,
        ):
            # ---- loads ----
            xg_sb = cp.tile([9, NGP], bf16, tag="xg")
            Wq = [cp.tile([9, 128], bf16, tag=f"Wq{s}", name=f"Wq{s}")
                  for s in range(NSETS)]
            eb = cp.tile([128, NSETS], fp32, tag="eb")
            lt_sb = [cp.tile([128, F], fp16, tag=f"lt{s}", name=f"lt{s}")
                     for s in range(NSETS)]
            oc_sb = [cp.tile([128, 256], bf16, tag=f"oc{t}", name=f"oc{t}")
                     for t in range(MTC)]
            ca = cp.tile([128, 4], fp32, tag="ca")
            ix = cp.tile([128, 4], int32, tag="ix")
            ident = cp.tile([128, 128], bf16, tag="ident")

            nc.sync.dma_start(xg_sb[:, 0:512], d_xg.ap()[:, 0:512])
            for s in range(NSETS):
                nc.sync.dma_start(Wq[s][:, :], d_wq.ap()[s])
            nc.sync.dma_start(xg_sb[:, 512:NGP], d_xg.ap()[:, 512:NGP])
            nc.gpsimd.dma_start(eb[:, :], d_eb.ap())
            for s in range(NSETS):
                nc.gpsimd.dma_start(lt_sb[s][:, :],
                                    d_lt.ap()[s * 128:(s + 1) * 128, :])
            nc.gpsimd.dma_start(ca[:, :], d_ca.ap())
            nc.gpsimd.dma_start(ix[:, :], d_ix.ap())
            nc.sync.dma_start(ident[:, :], d_id.ap())
            for t in range(MTC):
                nc.sync.dma_start(oc_sb[t][:, :],
                                  d_oc.ap()[t * 128:(t + 1) * 128, :])

            # ---- softmax numerators + Z ----
            elt = []
            for s in range(NSETS):
                e = cp.tile([128, F], fp16, tag=f"elt{s}", name=f"elt{s}")
                nc.scalar.activation(e[:, :], lt_sb[s][:, :], AF.Exp)
                elt.append(e)
            Wamp = []
            for s in range(NSETS):
                w = cp.tile([128, 512], fp16, tag=f"Wamp{s}", name=f"Wamp{s}")
                nc.vector.memset(w[:, :], 0.0)
                for h in range(2):
                    b = 2 * s + h
                    nc.vector.tensor_copy(
                        w[h * 64:(h + 1) * 64, b * 128:(b + 1) * 128],
                        elt[s][h * 64:(h + 1) * 64, :])
                Wamp.append(w)
            ones2 = cp.tile([128, 2], fp16, tag="ones2")
            nc.vector.memset(ones2[:, :], 0.0)
            nc.vector.memset(ones2[0:64, 0:1], 1.0)
            nc.vector.memset(ones2[64:128, 1:2], 1.0)
            Zc = cp.tile([128, 2 * NSETS], fp32, tag="Zc")
            for s in range(NSETS):
                zp = ps.tile([128, 512], fp32, tag="R", name=f"zp{s}")
                nc.tensor.matmul(zp[:, 0:2], elt[s][:, :], ones2[:, :],
                                 start=True, stop=True)
                nc.vector.tensor_copy(Zc[:, 2 * s:2 * s + 2], zp[:, 0:2])
            nc.sync.dma_start(d_z.ap(), Zc[:, :])

            # ---- K (all chunks), then per-b-block pipelined T build ->
            # DMA -> gather -> defrac -> transpose ----
            K_sb = [cp.tile([128, NGP], fp16, tag=f"K{s}", name=f"K{s}")
                    for s in range(NSETS)]
            T_sb = [cp.tile([128, NGP], bf16, tag=f"Tsb{t}", name=f"Tsb{t}")
                    for t in range(4)]
            Tw = cp.tile([128, 4 * RUNP], bf16, tag="Tw")
            T2 = cp.tile([128, 4 * MT], bf16, tag="T2")
            tmp = cp.tile([128, M2P], bf16, tag="tmp")
            T2t = [cp.tile([128, 512], bf16, tag=f"T2t{t}", name=f"T2t{t}")
                   for t in range(MTC)]
            nc.vector.memset(T2[:, :], 0.0)
            teng = [nc.sync, nc.scalar]
            ti = 0

            for c in range(NGC):
                for s in range(NSETS):
                    psK = ps.tile([128, 512], fp32, tag="K",
                                  name=f"psK{c}_{s}")
                    nc.tensor.matmul(psK[:, :], Wq[s][:, :],
                                     xg_sb[:, c * 512:(c + 1) * 512],
                                     start=True, stop=True)
                    nc.scalar.activation(K_sb[s][:, c * 512:(c + 1) * 512],
                                         psK[:, :], AF.Exp,
                                         scale=float(minus_c),
                                         bias=eb[:, s:s + 1])

            # all T matmuls + ACT drains + DMAs, then gathers chase them
            for t in range(4):
                for c in range(NGC):
                    psT = ps.tile([128, 512], fp32, tag="T", name=f"psT{c}_{t}")
                    for s in range(NSETS):
                        nc.tensor.matmul(
                            psT[:, :], Wamp[s][:, t * 128:(t + 1) * 128],
                            K_sb[s][:, c * 512:(c + 1) * 512],
                            start=(s == 0), stop=(s == NSETS - 1))
                    nc.scalar.activation(
                        T_sb[t][:, c * 512:(c + 1) * 512], psT[:, :], AF.Copy)
                    teng[ti % 2].dma_start(
                        bass.AP(tensor=d_t, offset=t * 128 * NGP + c * 512,
                                ap=[[NGP, 128], [1, 512]]),
                        T_sb[t][:, c * 512:(c + 1) * 512])
                    ti += 1
                # gather this block's per-f windows (one idx per out row)
                nc.gpsimd.indirect_dma_start(
                    out=Tw[:, t * RUNP:(t + 1) * RUNP],
                    out_offset=None,
                    in_=bass.AP(tensor=d_t, offset=0,
                                ap=[[1, t * 128 * NGP + 128 * NGP], [1, 1]]),
                    in_offset=bass.IndirectOffsetOnAxis(
                        ap=ix[:, t:t + 1], axis=0),
                )

            # defrac (DVE) -> transpose (PE) -> per-block final matmuls
            psO = [ps.tile([128, 512], fp32, tag="R", name=f"psO{vh}")
                   for vh in range(2)]
            for t in range(4):
                o = t * RUNP
                m = t * MT
                nc.vector.tensor_scalar(T2[:, m:m + M2P], Tw[:, o:o + M2P],
                                        ca[:, 0:1], None, op0=OP.mult)
                for r in range(1, 4):
                    nc.vector.tensor_scalar(tmp[:, :], Tw[:, o + r:o + r + M2P],
                                            ca[:, r:r + 1], None, op0=OP.mult)
                    nc.vector.tensor_tensor(T2[:, m:m + M2P], T2[:, m:m + M2P],
                                            tmp[:, :], OP.add)
                for mt in range(MTC):
                    psR = ps.tile([128, 128], bf16, tag="Rb",
                                  name=f"psR{t}_{mt}")
                    nc.tensor.transpose(
                        psR[:, :],
                        T2[:, t * MT + mt * 128:t * MT + (mt + 1) * 128],
                        ident[:, :])
                    nc.vector.tensor_copy(
                        T2t[mt][:, t * 128:(t + 1) * 128], psR[:, :])
                for vh in range(2):
                    for mt in range(MTC):
                        nc.tensor.matmul(
                            psO[vh][:, t * 128:(t + 1) * 128],
                            oc_sb[mt][:, vh * 128:(vh + 1) * 128],
                            T2t[mt][:, t * 128:(t + 1) * 128],
                            start=(mt == 0), stop=(mt == MTC - 1))

            # ---- drain outputs ----
            ob0 = cp.tile([128, 512], fp16, tag="ob0")
            nc.vector.tensor_copy(ob0[:, :], psO[0][:, :])
            nc.sync.dma_start(d_out.ap()[0], ob0[:, :])
            ob1 = cp.tile([128, 512], fp16, tag="ob1")
            nc.scalar.activation(ob1[:, :], psO[1][:, :], AF.Copy)
            nc.scalar.dma_start(d_out.ap()[1], ob1[:, :])

    nc.compile()
    return nc


def _get_nc(minus_c, NGP, MT, M2P, RUNP):
    key = (round(float(minus_c), 4), NGP, MT, M2P, RUNP)
    if key not in _cache:
        _cache[key] = _build(minus_c, NGP, MT, M2P, RUNP)
    return _cache[key]


def _split3(v):
    h = v.astype(BF16)
    r1 = v - h.astype(np.float32)
    m = r1.astype(BF16)
    r2 = r1 - m.astype(np.float32)
    return h.astype(np.float32), m.astype(np.float32), r2.astype(BF16)


def _cr(t):
    t2 = t * t
    t3 = t2 * t
    return np.stack([-0.5 * t3 + t2 - 0.5 * t, 1.5 * t3 - 2.5 * t2 + 1,
                     -1.5 * t3 + 2 * t2 + 0.5 * t, 0.5 * t3 - 0.5 * t2])


def kernel(q2_obs_scaled, amplitude_logits, volumes, filters, sigma,
           _trace=False, _tmpdir=None):
    from concourse.bass_utils import run_bass_kernel_spmd

    sig_eff = float(np.asarray(sigma).reshape(())) + 0.001
    minus_c = -0.5 / sig_eff ** 2

    q = np.ascontiguousarray(np.asarray(q2_obs_scaled, dtype=np.float32))
    lg = np.asarray(amplitude_logits, dtype=np.float32).reshape(B, F, P)
    vol = np.ascontiguousarray(np.asarray(volumes, dtype=np.float32).reshape(V))
    fil = np.ascontiguousarray(np.asarray(filters, dtype=np.float32).reshape(F))

    # ---- grid + interpolation geometry (host) ----
    x = fil[:, None] * vol[None, :]
    xmax = float(x.max())
    eps = EPS_FRAC * sig_eff / xmax
    x0 = float(x.min()) * 0.999 * np.exp(-8 * eps)
    NG = int(np.ceil(np.log(xmax / x0) / eps)) + 10
    NGP = ((NG + 511) // 512) * 512
    xg = (x0 * np.exp(eps * np.arange(NGP))).astype(np.float32)
    af = np.log(fil / x0) / eps
    cv = np.log(vol) / eps
    A = np.floor(af).astype(np.int64)
    alpha = (af - A).astype(np.float32)
    C = np.floor(cv).astype(np.int64)
    gamma = (cv - C).astype(np.float32)
    Mlo = int(C.min()) - 2
    M2 = int(C.max()) + 3 - Mlo
    M2P = ((M2 + 3) // 4) * 4
    RUNP = M2P + 4
    MT = ((M2P + 127) // 128) * 128

    wC = _cr(gamma)                                     # [4, V]
    OC = np.zeros((MT, 256), dtype=np.float32)
    for j in range(4):
        OC[(C - Mlo) + j - 1, np.arange(256)] += wC[j]
    OC = OC.astype(BF16)
    caw = _cr(alpha).T.astype(np.float32)               # [128, 4]
    # gather start: flat index of T[rfb, A_f + Mlo - 1]
    pidx = np.arange(128)
    ixw = np.empty((128, 4), dtype=np.int32)
    for blk in range(4):
        ixw[:, blk] = (blk * 128 + pidx) * NGP + A[pidx] + Mlo - 1
    assert ixw.min() >= 0 and ixw.max() + RUNP <= 4 * 128 * NGP

    ah, am, al = _split3(xg * xg)
    bh, bm, bl = _split3(xg)
    xgp = np.stack([ah, am, al, bh, bh, bh, bm, bm, bl]).astype(BF16)

    nc = _get_nc(minus_c, NGP, MT, M2P, RUNP)

    idm = np.eye(128, dtype=BF16)
    in_maps = []
    for i in range(NCORES):
        bsl = slice(i * B_LOC, (i + 1) * B_LOC)
        qc = q[bsl].reshape(B_LOC * P)
        lgc = lg[bsl]
        wq = np.empty((NSETS, 9, 128), dtype=BF16)
        ebias = np.empty((128, NSETS), dtype=np.float32)
        for s in range(NSETS):
            qs = qc[s * 128:(s + 1) * 128]
            wh, wm, wl = _split3(-2.0 * qs)
            one = np.ones(128, dtype=np.float32)
            wq[s] = np.stack([one, one, one, wh, wm, wl, wh, wm, wh]
                             ).astype(BF16)
            ebias[:, s] = minus_c * qs * qs
        in_maps.append({
            "xg": xgp, "wq": wq, "eb": ebias,
            "lt": np.ascontiguousarray(
                lgc.transpose(0, 2, 1).reshape(B_LOC * P, F)
            ).astype(np.float16),
            "oc": OC, "ca": caw, "ix": ixw, "idm": idm,
        })

    kw = {}
    if _trace:
        kw = {"trace": True, "tmpdir": _tmpdir}
    res = run_bass_kernel_spmd(nc, in_maps, core_ids=list(range(NCORES)), **kw)

    out = np.empty((B, V, F), dtype=np.float32)
    for i in range(NCORES):
        oc = res.results[i]["out"].astype(np.float32)   # (2, 128, 512)
        zc = res.results[i]["zout"]                     # (128 f, 4 b)
        for b in range(B_LOC):
            for vh in range(2):
                out[i * B_LOC + b, vh * 128:(vh + 1) * 128, :] \
                    = oc[vh, :, b * 128:(b + 1) * 128] / zc[:, b][None, :]
    if _trace:
        return out, res
    return out


# revision 14
# speedup vs baseline: 2.7938x; 1.0735x over previous
"""Trainium2 Bass kernel for nn_ExtractionLayer — log-grid table algorithm.

metric[b,v,f] = sum_p amp[b,f,p] * G(x_vf - q_bp),  x_vf = fil_f*vol_v,
G = Gaussian of width sig_eff = sigma+0.001, amp = softmax_p(logits).

Instead of 8.4M exps/core (direct method), build the amp-weighted Gaussian
mixture h_{b,f} on a LOG-SPACED x-grid (NG~1.5K points; exps = NG*256 only)
and evaluate metric by two nested cubic (Catmull-Rom) interpolations:
  position(x) = ln(x/x0)/eps = a_f + c_v   (separable in log space!)
  stage alpha: per-(f,b) window gather of T rows (indirect DMA) + fractional
               defrac with per-partition cubic weights (DVE)
  stage gamma: dense matmul with a host-built interp matrix OC[m, v].

Per-core flow (4 b's/core, data-parallel over batch):
  1. elt=exp(lt) (ACT), block-diag Wamp (DVE), Z via PE ones-matmul (host
     divides output by Z).
  2. K[(bp), g] = exp(-c*(x_g - q)^2) via the 9-row bf16-split matmul + ACT,
     g in NGP cols.  (~1.4K*256 exps = ~3us)
  3. T[rfb, g] = sum_p amp*K: PE matmuls, lhsT = Wamp cols, ACCUMULATED over
     both (bp) set-tiles; rfb = b*128+f on partitions. -> bf16 -> DRAM.
  4. gather: ONE indirect DMA pulls per-(f,b) windows T[rfb, A_f+Mlo-1 : +RUN]
     (coef=1 flat indexing, 512 descriptors).
  5. defrac (stage alpha): T2[rfb, m] = sum_r ca_r(alpha_f)*Tw[rfb, m+r].
  6. PE transpose T2 -> [m-part, rfb], then out[v, rfb] = OC^T @ T2t
     accumulated over m-tiles. -> fp16 -> DRAM out.
Host packs: grid splits, Wq/ebias, OC, defrac coeffs, gather indices.
"""

import sys

for _p in ("/opt/trn_rl_repo", "/root/.axon_site/_ro/trn_rl_repo"):
    if _p not in sys.path:
        sys.path.append(_p)

import numpy as np
import ml_dtypes

BF16 = ml_dtypes.bfloat16

B, V, F, P = 32, 256, 128, 64
NCORES = 8
B_LOC = B // NCORES
NSETS = B_LOC // 2
EPS_FRAC = 0.9

_cache: dict = {}


def _build(minus_c, NGP, MT, M2P, RUNP):
    import concourse.tile as tile
    from concourse import bacc, mybir

    fp32 = mybir.dt.float32
    fp16 = mybir.dt.float16
    bf16 = mybir.dt.bfloat16
    int32 = mybir.dt.int32
    AF = mybir.ActivationFunctionType
    OP = mybir.AluOpType
    import concourse.bass as bass

    NGC = NGP // 512            # K/T column chunks
    MTC = MT // 128             # m partition-tiles

    nc = bacc.Bacc("TRN2", target_bir_lowering=False, debug=False,
                   num_devices=NCORES)

    d_xg = nc.dram_tensor("xg", [9, NGP], bf16, kind="ExternalInput")
    d_wq = nc.dram_tensor("wq", [NSETS, 9, 128], bf16, kind="ExternalInput")
    d_eb = nc.dram_tensor("eb", [128, NSETS], fp32, kind="ExternalInput")
    d_lt = nc.dram_tensor("lt", [B_LOC * P, F], fp16, kind="ExternalInput")
    d_oc = nc.dram_tensor("oc", [MT, 256], bf16, kind="ExternalInput")
    d_ca = nc.dram_tensor("ca", [128, 4], fp32, kind="ExternalInput")
    d_ix = nc.dram_tensor("ix", [128, 4], int32, kind="ExternalInput")
    d_id = nc.dram_tensor("idm", [128, 128], bf16, kind="ExternalInput")
    d_t = nc.dram_tensor("tsc", [4 * 128 * NGP], bf16)     # T scratch, flat
    d_out = nc.dram_tensor("out", [2, 128, 512], fp16, kind="ExternalOutput")
    d_z = nc.dram_tensor("zout", [128, 2 * NSETS], fp32, kind="ExternalOutput")

    with tile.TileContext(nc) as tc:
        with (
            tc.tile_pool(name="const", bufs=1) as cp,
            tc.tile_pool(name="ps", bufs=2, space=bass.MemorySpace.PSUM) as ps,
        ):
            # ---- loads ----
            xg_sb = cp.tile([9, NGP], bf16, tag="xg")
            Wq = [cp.tile([9, 128], bf16, tag=f"Wq{s}", name=f"Wq{s}")
                  for s in range(NSETS)]
            eb = cp.tile([128, NSETS], fp32, tag="eb")
            lt_sb = [cp.tile([128, F], fp16, tag=f"lt{s}", name=f"lt{s}")
                     for s in range(NSETS)]
            oc_sb = [cp.tile([128, 256], bf16, tag=f"oc{t}", name=f"oc{t}")
                     for t in range(MTC)]
            ca = cp.tile([128, 4], fp32, tag="ca")
            ix = cp.tile([128, 4], int32, tag="ix")
            ident = cp.tile([128, 128], bf16, tag="ident")

            for s in range(NSETS):
                nc.sync.dma_start(Wq[s][:, :], d_wq.ap()[s])
            nc.sync.dma_start(xg_sb[:, 0:512], d_xg.ap()[:, 0:512])
            nc.sync.dma_start(xg_sb[:, 512:NGP], d_xg.ap()[:, 512:NGP])
            nc.gpsimd.dma_start(eb[:, :], d_eb.ap())
            for s in range(NSETS):
                nc.gpsimd.dma_start(lt_sb[s][:, :],
                                    d_lt.ap()[s * 128:(s + 1) * 128, :])
            nc.gpsimd.dma_start(ca[:, :], d_ca.ap())
            nc.gpsimd.dma_start(ix[:, :], d_ix.ap())
            nc.sync.dma_start(ident[:, :], d_id.ap())
            for t in range(MTC):
                nc.sync.dma_start(oc_sb[t][:, :],
                                  d_oc.ap()[t * 128:(t + 1) * 128, :])

            # ---- softmax numerators + Z ----
            elt = []
            for s in range(NSETS):
                e = cp.tile([128, F], fp16, tag=f"elt{s}", name=f"elt{s}")
                nc.scalar.activation(e[:, :], lt_sb[s][:, :], AF.Exp)
                elt.append(e)
            Wamp = []
            for s in range(NSETS):
                w = cp.tile([128, 512], fp16, tag=f"Wamp{s}", name=f"Wamp{s}")
                nc.vector.memset(w[:, :], 0.0)
                for h in range(2):
                    b = 2 * s + h
                    nc.vector.tensor_copy(
                        w[h * 64:(h + 1) * 64, b * 128:(b + 1) * 128],
                        elt[s][h * 64:(h + 1) * 64, :])
                Wamp.append(w)
            ones2 = cp.tile([128, 2], fp16, tag="ones2")
            nc.vector.memset(ones2[:, :], 0.0)
            nc.vector.memset(ones2[0:64, 0:1], 1.0)
            nc.vector.memset(ones2[64:128, 1:2], 1.0)
            Zc = cp.tile([128, 2 * NSETS], fp32, tag="Zc")
            for s in range(NSETS):
                zp = ps.tile([128, 512], fp32, tag="R", name=f"zp{s}")
                nc.tensor.matmul(zp[:, 0:2], elt[s][:, :], ones2[:, :],
                                 start=True, stop=True)
                nc.vector.tensor_copy(Zc[:, 2 * s:2 * s + 2], zp[:, 0:2])
            nc.sync.dma_start(d_z.ap(), Zc[:, :])

            # ---- K (all chunks), then per-b-block pipelined T build ->
            # DMA -> gather -> defrac -> transpose ----
            K_sb = [cp.tile([128, NGP], fp16, tag=f"K{s}", name=f"K{s}")
                    for s in range(NSETS)]
            T_sb = [cp.tile([128, NGP], bf16, tag=f"Tsb{t}", name=f"Tsb{t}")
                    for t in range(4)]
            Tw = cp.tile([128, 4 * RUNP], bf16, tag="Tw")
            T2 = cp.tile([128, 4 * MT], bf16, tag="T2")
            tmp = cp.tile([128, M2P], bf16, tag="tmp")
            T2t = [cp.tile([128, 512], bf16, tag=f"T2t{t}", name=f"T2t{t}")
                   for t in range(MTC)]
            nc.vector.memset(T2[:, :], 0.0)
            teng = [nc.sync, nc.scalar]
            ti = 0

            for c in range(NGC):
                for s in range(NSETS):
                    psK = ps.tile([128, 512], fp32, tag="K",
                                  name=f"psK{c}_{s}")
                    nc.tensor.matmul(psK[:, :], Wq[s][:, :],
                                     xg_sb[:, c * 512:(c + 1) * 512],
                                     start=True, stop=True)
                    nc.scalar.activation(K_sb[s][:, c * 512:(c + 1) * 512],
                                         psK[:, :], AF.Exp,
                                         scale=float(minus_c),
                                         bias=eb[:, s:s + 1])

            # all T matmuls + ACT drains + DMAs, then gathers chase them
            for t in range(4):
                for c in range(NGC):
                    psT = ps.tile([128, 512], fp32, tag="T", name=f"psT{c}_{t}")
                    for s in range(NSETS):
                        nc.tensor.matmul(
                            psT[:, :], Wamp[s][:, t * 128:(t + 1) * 128],
                            K_sb[s][:, c * 512:(c + 1) * 512],
                            start=(s == 0), stop=(s == NSETS - 1))
                    if ti % 2 == 0:
                        nc.vector.tensor_copy(
                            T_sb[t][:, c * 512:(c + 1) * 512], psT[:, :])
                    else:
                        nc.scalar.activation(
                            T_sb[t][:, c * 512:(c + 1) * 512], psT[:, :],
                            AF.Copy)
                    teng[ti % 2].dma_start(
                        bass.AP(tensor=d_t, offset=t * 128 * NGP + c * 512,
                                ap=[[NGP, 128], [1, 512]]),
                        T_sb[t][:, c * 512:(c + 1) * 512])
                    ti += 1
                # gather this block's per-f windows (one idx per out row)
                nc.gpsimd.indirect_dma_start(
                    out=Tw[:, t * RUNP:(t + 1) * RUNP],
                    out_offset=None,
                    in_=bass.AP(tensor=d_t, offset=0,
                                ap=[[1, t * 128 * NGP + 128 * NGP], [1, 1]]),
                    in_offset=bass.IndirectOffsetOnAxis(
                        ap=ix[:, t:t + 1], axis=0),
                )

            # defrac (DVE) -> transpose (PE) -> per-block final matmuls
            psO = [ps.tile([128, 512], fp32, tag="R", name=f"psO{vh}")
                   for vh in range(2)]
            tmps = [cp.tile([128, M2P], bf16, tag=f"tm{r}", name=f"tm{r}")
                    for r in range(3)]
            for t in range(4):
                o = t * RUNP
                m = t * MT
                nc.scalar.activation(T2[:, m:m + M2P], Tw[:, o:o + M2P],
                                     AF.Copy, scale=ca[:, 0:1])
                for r in range(1, 4):
                    nc.scalar.activation(tmps[r - 1][:, :],
                                         Tw[:, o + r:o + r + M2P],
                                         AF.Copy, scale=ca[:, r:r + 1])
                for r in range(1, 4):
                    nc.vector.tensor_tensor(T2[:, m:m + M2P], T2[:, m:m + M2P],
                                            tmps[r - 1][:, :], OP.add)
                for mt in range(MTC):
                    psR = ps.tile([128, 128], bf16, tag="Rb",
                                  name=f"psR{t}_{mt}")
                    nc.tensor.transpose(
                        psR[:, :],
                        T2[:, t * MT + mt * 128:t * MT + (mt + 1) * 128],
                        ident[:, :])
                    nc.vector.tensor_copy(
                        T2t[mt][:, t * 128:(t + 1) * 128], psR[:, :])
                for vh in range(2):
                    for mt in range(MTC):
                        nc.tensor.matmul(
                            psO[vh][:, t * 128:(t + 1) * 128],
                            oc_sb[mt][:, vh * 128:(vh + 1) * 128],
                            T2t[mt][:, t * 128:(t + 1) * 128],
                            start=(mt == 0), stop=(mt == MTC - 1))

            # ---- drain outputs ----
            ob0 = cp.tile([128, 512], fp16, tag="ob0")
            nc.vector.tensor_copy(ob0[:, :], psO[0][:, :])
            nc.sync.dma_start(d_out.ap()[0], ob0[:, :])
            ob1 = cp.tile([128, 512], fp16, tag="ob1")
            nc.scalar.activation(ob1[:, :], psO[1][:, :], AF.Copy)
            nc.scalar.dma_start(d_out.ap()[1], ob1[:, :])

    nc.compile()
    return nc


def _get_nc(minus_c, NGP, MT, M2P, RUNP):
    key = (round(float(minus_c), 4), NGP, MT, M2P, RUNP)
    if key not in _cache:
        _cache[key] = _build(minus_c, NGP, MT, M2P, RUNP)
    return _cache[key]


def _split3(v):
    h = v.astype(BF16)
    r1 = v - h.astype(np.float32)
    m = r1.astype(BF16)
    r2 = r1 - m.astype(np.float32)
    return h.astype(np.float32), m.astype(np.float32), r2.astype(BF16)


def _cr(t):
    t2 = t * t
    t3 = t2 * t
    return np.stack([-0.5 * t3 + t2 - 0.5 * t, 1.5 * t3 - 2.5 * t2 + 1,
                     -1.5 * t3 + 2 * t2 + 0.5 * t, 0.5 * t3 - 0.5 * t2])


def kernel(q2_obs_scaled, amplitude_logits, volumes, filters, sigma,
           _trace=False, _tmpdir=None):
    from concourse.bass_utils import run_bass_kernel_spmd

    sig_eff = float(np.asarray(sigma).reshape(())) + 0.001
    minus_c = -0.5 / sig_eff ** 2

    q = np.ascontiguousarray(np.asarray(q2_obs_scaled, dtype=np.float32))
    lg = np.asarray(amplitude_logits, dtype=np.float32).reshape(B, F, P)
    vol = np.ascontiguousarray(np.asarray(volumes, dtype=np.float32).reshape(V))
    fil = np.ascontiguousarray(np.asarray(filters, dtype=np.float32).reshape(F))

    # ---- grid + interpolation geometry (host) ----
    x = fil[:, None] * vol[None, :]
    xmax = float(x.max())
    eps = EPS_FRAC * sig_eff / xmax
    x0 = float(x.min()) * 0.999 * np.exp(-8 * eps)
    NG = int(np.ceil(np.log(xmax / x0) / eps)) + 10
    NGP = ((NG + 511) // 512) * 512
    xg = (x0 * np.exp(eps * np.arange(NGP))).astype(np.float32)
    af = np.log(fil / x0) / eps
    cv = np.log(vol) / eps
    A = np.floor(af).astype(np.int64)
    alpha = (af - A).astype(np.float32)
    C = np.floor(cv).astype(np.int64)
    gamma = (cv - C).astype(np.float32)
    Mlo = int(C.min()) - 2
    M2 = int(C.max()) + 3 - Mlo
    M2P = ((M2 + 3) // 4) * 4
    RUNP = M2P + 4
    MT = ((M2P + 127) // 128) * 128

    wC = _cr(gamma)                                     # [4, V]
    OC = np.zeros((MT, 256), dtype=np.float32)
    for j in range(4):
        OC[(C - Mlo) + j - 1, np.arange(256)] += wC[j]
    OC = OC.astype(BF16)
    caw = _cr(alpha).T.astype(np.float32)               # [128, 4]
    # gather start: flat index of T[rfb, A_f + Mlo - 1]
    pidx = np.arange(128)
    ixw = np.empty((128, 4), dtype=np.int32)
    for blk in range(4):
        ixw[:, blk] = (blk * 128 + pidx) * NGP + A[pidx] + Mlo - 1
    assert ixw.min() >= 0 and ixw.max() + RUNP <= 4 * 128 * NGP

    ah, am, al = _split3(xg * xg)
    bh, bm, bl = _split3(xg)
    xgp = np.stack([ah, am, al, bh, bh, bh, bm, bm, bl]).astype(BF16)

    nc = _get_nc(minus_c, NGP, MT, M2P, RUNP)

    idm = np.eye(128, dtype=BF16)
    in_maps = []
    for i in range(NCORES):
        bsl = slice(i * B_LOC, (i + 1) * B_LOC)
        qc = q[bsl].reshape(B_LOC * P)
        lgc = lg[bsl]
        wq = np.empty((NSETS, 9, 128), dtype=BF16)
        ebias = np.empty((128, NSETS), dtype=np.float32)
        for s in range(NSETS):
            qs = qc[s * 128:(s + 1) * 128]
            wh, wm, wl = _split3(-2.0 * qs)
            one = np.ones(128, dtype=np.float32)
            wq[s] = np.stack([one, one, one, wh, wm, wl, wh, wm, wh]
                             ).astype(BF16)
            ebias[:, s] = minus_c * qs * qs
        in_maps.append({
            "xg": xgp, "wq": wq, "eb": ebias,
            "lt": np.ascontiguousarray(
                lgc.transpose(0, 2, 1).reshape(B_LOC * P, F)
            ).astype(np.float16),
            "oc": OC, "ca": caw, "ix": ixw, "idm": idm,
        })

    kw = {}
    if _trace:
        kw = {"trace": True, "tmpdir": _tmpdir}
    res = run_bass_kernel_spmd(nc, in_maps, core_ids=list(range(NCORES)), **kw)

    out = np.empty((B, V, F), dtype=np.float32)
    for i in range(NCORES):
        oc = res.results[i]["out"].astype(np.float32)   # (2, 128, 512)
        zc = res.results[i]["zout"]                     # (128 f, 4 b)
        for b in range(B_LOC):
            for vh in range(2):
                out[i * B_LOC + b, vh * 128:(vh + 1) * 128, :] \
                    = oc[vh, :, b * 128:(b + 1) * 128] / zc[:, b][None, :]
    if _trace:
        return out, res
    return out


# revision 15
# speedup vs baseline: 2.8580x; 1.0230x over previous
"""Trainium2 Bass kernel for nn_ExtractionLayer — log-grid table algorithm.

metric[b,v,f] = sum_p amp[b,f,p] * G(x_vf - q_bp),  x_vf = fil_f*vol_v,
G = Gaussian of width sig_eff = sigma+0.001, amp = softmax_p(logits).

Instead of 8.4M exps/core (direct method), build the amp-weighted Gaussian
mixture h_{b,f} on a LOG-SPACED x-grid (NG~1.5K points; exps = NG*256 only)
and evaluate metric by two nested cubic (Catmull-Rom) interpolations:
  position(x) = ln(x/x0)/eps = a_f + c_v   (separable in log space!)
  stage alpha: per-(f,b) window gather of T rows (indirect DMA) + fractional
               defrac with per-partition cubic weights (DVE)
  stage gamma: dense matmul with a host-built interp matrix OC[m, v].

Per-core flow (4 b's/core, data-parallel over batch):
  1. elt=exp(lt) (ACT), block-diag Wamp (DVE), Z via PE ones-matmul (host
     divides output by Z).
  2. K[(bp), g] = exp(-c*(x_g - q)^2) via the 9-row bf16-split matmul + ACT,
     g in NGP cols.  (~1.4K*256 exps = ~3us)
  3. T[rfb, g] = sum_p amp*K: PE matmuls, lhsT = Wamp cols, ACCUMULATED over
     both (bp) set-tiles; rfb = b*128+f on partitions. -> bf16 -> DRAM.
  4. gather: ONE indirect DMA pulls per-(f,b) windows T[rfb, A_f+Mlo-1 : +RUN]
     (coef=1 flat indexing, 512 descriptors).
  5. defrac (stage alpha): T2[rfb, m] = sum_r ca_r(alpha_f)*Tw[rfb, m+r].
  6. PE transpose T2 -> [m-part, rfb], then out[v, rfb] = OC^T @ T2t
     accumulated over m-tiles. -> fp16 -> DRAM out.
Host packs: grid splits, Wq/ebias, OC, defrac coeffs, gather indices.
"""

import sys

for _p in ("/opt/trn_rl_repo", "/root/.axon_site/_ro/trn_rl_repo"):
    if _p not in sys.path:
        sys.path.append(_p)

import numpy as np
import ml_dtypes

BF16 = ml_dtypes.bfloat16

B, V, F, P = 32, 256, 128, 64
NCORES = 8
B_LOC = B // NCORES
NSETS = B_LOC // 2
EPS_FRAC = 0.9

_cache: dict = {}


def _build(minus_c, NGP, MT, M2P, RUNP):
    import concourse.tile as tile
    from concourse import bacc, mybir

    fp32 = mybir.dt.float32
    fp16 = mybir.dt.float16
    bf16 = mybir.dt.bfloat16
    int32 = mybir.dt.int32
    AF = mybir.ActivationFunctionType
    OP = mybir.AluOpType
    import concourse.bass as bass

    NGC = NGP // 512            # K/T column chunks
    MTC = MT // 128             # m partition-tiles

    nc = bacc.Bacc("TRN2", target_bir_lowering=False, debug=False,
                   num_devices=NCORES)

    d_xg = nc.dram_tensor("xg", [9, NGP], bf16, kind="ExternalInput")
    d_wq = nc.dram_tensor("wq", [NSETS, 9, 128], bf16, kind="ExternalInput")
    d_eb = nc.dram_tensor("eb", [128, NSETS], fp32, kind="ExternalInput")
    d_lt = nc.dram_tensor("lt", [B_LOC * P, F], fp16, kind="ExternalInput")
    d_oc = nc.dram_tensor("oc", [MT, 256], bf16, kind="ExternalInput")
    d_ca = nc.dram_tensor("ca", [128, 4], fp32, kind="ExternalInput")
    d_ix = nc.dram_tensor("ix", [128, 4], int32, kind="ExternalInput")
    d_id = nc.dram_tensor("idm", [128, 128], bf16, kind="ExternalInput")
    d_t = nc.dram_tensor("tsc", [4 * 128 * NGP], bf16)     # T scratch, flat
    d_out = nc.dram_tensor("out", [2, 128, 512], fp16, kind="ExternalOutput")
    d_z = nc.dram_tensor("zout", [128, 2 * NSETS], fp32, kind="ExternalOutput")

    with tile.TileContext(nc) as tc:
        with (
            tc.tile_pool(name="const", bufs=1) as cp,
            tc.tile_pool(name="ps", bufs=2, space=bass.MemorySpace.PSUM) as ps,
        ):
            # ---- loads ----
            xg_sb = cp.tile([9, NGP], bf16, tag="xg")
            Wq = [cp.tile([9, 128], bf16, tag=f"Wq{s}", name=f"Wq{s}")
                  for s in range(NSETS)]
            eb = cp.tile([128, NSETS], fp32, tag="eb")
            lt_sb = [cp.tile([128, F], fp16, tag=f"lt{s}", name=f"lt{s}")
                     for s in range(NSETS)]
            oc_sb = [cp.tile([128, 256], bf16, tag=f"oc{t}", name=f"oc{t}")
                     for t in range(MTC)]
            ca = cp.tile([128, 4], fp32, tag="ca")
            ix = cp.tile([128, 4], int32, tag="ix")
            ident = cp.tile([128, 128], bf16, tag="ident")

            for s in range(NSETS):
                nc.sync.dma_start(Wq[s][:, :], d_wq.ap()[s])
            nc.sync.dma_start(xg_sb[:, 0:512], d_xg.ap()[:, 0:512])
            nc.sync.dma_start(xg_sb[:, 512:NGP], d_xg.ap()[:, 512:NGP])
            nc.gpsimd.dma_start(eb[:, :], d_eb.ap())
            for s in range(NSETS):
                nc.gpsimd.dma_start(lt_sb[s][:, :],
                                    d_lt.ap()[s * 128:(s + 1) * 128, :])
            nc.gpsimd.dma_start(ca[:, :], d_ca.ap())
            nc.gpsimd.dma_start(ix[:, :], d_ix.ap())
            nc.sync.dma_start(ident[:, :], d_id.ap())
            for t in range(MTC):
                nc.sync.dma_start(oc_sb[t][:, :],
                                  d_oc.ap()[t * 128:(t + 1) * 128, :])

            # ---- softmax numerators + Z ----
            elt = []
            for s in range(NSETS):
                e = cp.tile([128, F], fp16, tag=f"elt{s}", name=f"elt{s}")
                nc.scalar.activation(e[:, :], lt_sb[s][:, :], AF.Exp)
                elt.append(e)
            Wamp = []
            for s in range(NSETS):
                w = cp.tile([128, 512], fp16, tag=f"Wamp{s}", name=f"Wamp{s}")
                nc.vector.memset(w[:, :], 0.0)
                for h in range(2):
                    b = 2 * s + h
                    nc.vector.tensor_copy(
                        w[h * 64:(h + 1) * 64, b * 128:(b + 1) * 128],
                        elt[s][h * 64:(h + 1) * 64, :])
                Wamp.append(w)
            ones2 = cp.tile([128, 2], fp16, tag="ones2")
            nc.vector.memset(ones2[:, :], 0.0)
            nc.vector.memset(ones2[0:64, 0:1], 1.0)
            nc.vector.memset(ones2[64:128, 1:2], 1.0)
            Zc = cp.tile([128, 2 * NSETS], fp32, tag="Zc")
            for s in range(NSETS):
                zp = ps.tile([128, 512], fp32, tag="R", name=f"zp{s}")
                nc.tensor.matmul(zp[:, 0:2], elt[s][:, :], ones2[:, :],
                                 start=True, stop=True)
                nc.vector.tensor_copy(Zc[:, 2 * s:2 * s + 2], zp[:, 0:2])
            nc.sync.dma_start(d_z.ap(), Zc[:, :])

            # ---- K (all chunks), then per-b-block pipelined T build ->
            # DMA -> gather -> defrac -> transpose ----
            K_sb = [cp.tile([128, NGP], fp16, tag=f"K{s}", name=f"K{s}")
                    for s in range(NSETS)]
            T_sb = [cp.tile([128, NGP], bf16, tag=f"Tsb{t}", name=f"Tsb{t}")
                    for t in range(4)]
            Tw = cp.tile([128, 4 * RUNP], bf16, tag="Tw")
            T2 = cp.tile([128, 4 * MT], bf16, tag="T2")
            tmp = cp.tile([128, M2P], bf16, tag="tmp")
            T2t = [cp.tile([128, 512], bf16, tag=f"T2t{t}", name=f"T2t{t}")
                   for t in range(MTC)]
            nc.vector.memset(T2[:, :], 0.0)
            teng = [nc.sync, nc.scalar]
            ti = 0

            for c in range(NGC):
                for s in range(NSETS):
                    psK = ps.tile([128, 512], fp32, tag="K",
                                  name=f"psK{c}_{s}")
                    nc.tensor.matmul(psK[:, :], Wq[s][:, :],
                                     xg_sb[:, c * 512:(c + 1) * 512],
                                     start=True, stop=True)
                    nc.scalar.activation(K_sb[s][:, c * 512:(c + 1) * 512],
                                         psK[:, :], AF.Exp,
                                         scale=float(minus_c),
                                         bias=eb[:, s:s + 1])

            # all T matmuls + ACT drains + DMAs, then gathers chase them
            for t in range(4):
                for c in range(NGC):
                    psT = ps.tile([128, 512], fp32, tag="T", name=f"psT{c}_{t}")
                    for s in range(NSETS):
                        nc.tensor.matmul(
                            psT[:, :], Wamp[s][:, t * 128:(t + 1) * 128],
                            K_sb[s][:, c * 512:(c + 1) * 512],
                            start=(s == 0), stop=(s == NSETS - 1))
                    if ti % 2 == 0:
                        nc.vector.tensor_copy(
                            T_sb[t][:, c * 512:(c + 1) * 512], psT[:, :])
                    else:
                        nc.scalar.activation(
                            T_sb[t][:, c * 512:(c + 1) * 512], psT[:, :],
                            AF.Copy)
                    teng[ti % 2].dma_start(
                        bass.AP(tensor=d_t, offset=t * 128 * NGP + c * 512,
                                ap=[[NGP, 128], [1, 512]]),
                        T_sb[t][:, c * 512:(c + 1) * 512])
                    ti += 1
                # gather this block's per-f windows (one idx per out row)
                nc.gpsimd.indirect_dma_start(
                    out=Tw[:, t * RUNP:(t + 1) * RUNP],
                    out_offset=None,
                    in_=bass.AP(tensor=d_t, offset=0,
                                ap=[[1, t * 128 * NGP + 128 * NGP], [1, 1]]),
                    in_offset=bass.IndirectOffsetOnAxis(
                        ap=ix[:, t:t + 1], axis=0),
                )

            # defrac (DVE) -> transpose (PE) -> per-block final matmuls
            psO = [ps.tile([128, 512], fp32, tag="R", name=f"psO{vh}")
                   for vh in range(2)]
            tmps = [cp.tile([128, M2P], bf16, tag=f"tm{r}", name=f"tm{r}")
                    for r in range(3)]
            for t in range(4):
                o = t * RUNP
                m = t * MT
                nc.vector.tensor_scalar(T2[:, m:m + M2P], Tw[:, o:o + M2P],
                                        ca[:, 0:1], None, op0=OP.mult)
                nc.scalar.activation(tmps[0][:, :], Tw[:, o + 1:o + 1 + M2P],
                                     AF.Copy, scale=ca[:, 1:2])
                nc.scalar.activation(tmps[1][:, :], Tw[:, o + 2:o + 2 + M2P],
                                     AF.Copy, scale=ca[:, 2:3])
                nc.vector.tensor_scalar(tmps[2][:, :], Tw[:, o + 3:o + 3 + M2P],
                                        ca[:, 3:4], None, op0=OP.mult)
                for r in range(1, 4):
                    nc.vector.tensor_tensor(T2[:, m:m + M2P], T2[:, m:m + M2P],
                                            tmps[r - 1][:, :], OP.add)
                for mt in range(MTC):
                    psR = ps.tile([128, 128], bf16, tag="Rb",
                                  name=f"psR{t}_{mt}")
                    nc.tensor.transpose(
                        psR[:, :],
                        T2[:, t * MT + mt * 128:t * MT + (mt + 1) * 128],
                        ident[:, :])
                    nc.vector.tensor_copy(
                        T2t[mt][:, t * 128:(t + 1) * 128], psR[:, :])
                for vh in range(2):
                    for mt in range(MTC):
                        nc.tensor.matmul(
                            psO[vh][:, t * 128:(t + 1) * 128],
                            oc_sb[mt][:, vh * 128:(vh + 1) * 128],
                            T2t[mt][:, t * 128:(t + 1) * 128],
                            start=(mt == 0), stop=(mt == MTC - 1))

            # ---- drain outputs ----
            ob0 = cp.tile([128, 512], fp16, tag="ob0")
            nc.vector.tensor_copy(ob0[:, :], psO[0][:, :])
            nc.sync.dma_start(d_out.ap()[0], ob0[:, :])
            ob1 = cp.tile([128, 512], fp16, tag="ob1")
            nc.scalar.activation(ob1[:, :], psO[1][:, :], AF.Copy)
            nc.scalar.dma_start(d_out.ap()[1], ob1[:, :])

    nc.compile()
    return nc


def _get_nc(minus_c, NGP, MT, M2P, RUNP):
    key = (round(float(minus_c), 4), NGP, MT, M2P, RUNP)
    if key not in _cache:
        _cache[key] = _build(minus_c, NGP, MT, M2P, RUNP)
    return _cache[key]


def _split3(v):
    h = v.astype(BF16)
    r1 = v - h.astype(np.float32)
    m = r1.astype(BF16)
    r2 = r1 - m.astype(np.float32)
    return h.astype(np.float32), m.astype(np.float32), r2.astype(BF16)


def _cr(t):
    t2 = t * t
    t3 = t2 * t
    return np.stack([-0.5 * t3 + t2 - 0.5 * t, 1.5 * t3 - 2.5 * t2 + 1,
                     -1.5 * t3 + 2 * t2 + 0.5 * t, 0.5 * t3 - 0.5 * t2])


def kernel(q2_obs_scaled, amplitude_logits, volumes, filters, sigma,
           _trace=False, _tmpdir=None):
    from concourse.bass_utils import run_bass_kernel_spmd

    sig_eff = float(np.asarray(sigma).reshape(())) + 0.001
    minus_c = -0.5 / sig_eff ** 2

    q = np.ascontiguousarray(np.asarray(q2_obs_scaled, dtype=np.float32))
    lg = np.asarray(amplitude_logits, dtype=np.float32).reshape(B, F, P)
    vol = np.ascontiguousarray(np.asarray(volumes, dtype=np.float32).reshape(V))
    fil = np.ascontiguousarray(np.asarray(filters, dtype=np.float32).reshape(F))

    # ---- grid + interpolation geometry (host) ----
    x = fil[:, None] * vol[None, :]
    xmax = float(x.max())
    eps = EPS_FRAC * sig_eff / xmax
    x0 = float(x.min()) * 0.999 * np.exp(-8 * eps)
    NG = int(np.ceil(np.log(xmax / x0) / eps)) + 10
    NGP = ((NG + 511) // 512) * 512
    xg = (x0 * np.exp(eps * np.arange(NGP))).astype(np.float32)
    af = np.log(fil / x0) / eps
    cv = np.log(vol) / eps
    A = np.floor(af).astype(np.int64)
    alpha = (af - A).astype(np.float32)
    C = np.floor(cv).astype(np.int64)
    gamma = (cv - C).astype(np.float32)
    Mlo = int(C.min()) - 2
    M2 = int(C.max()) + 3 - Mlo
    M2P = ((M2 + 3) // 4) * 4
    RUNP = M2P + 4
    MT = ((M2P + 127) // 128) * 128

    wC = _cr(gamma)                                     # [4, V]
    OC = np.zeros((MT, 256), dtype=np.float32)
    for j in range(4):
        OC[(C - Mlo) + j - 1, np.arange(256)] += wC[j]
    OC = OC.astype(BF16)
    caw = _cr(alpha).T.astype(np.float32)               # [128, 4]
    # gather start: flat index of T[rfb, A_f + Mlo - 1]
    pidx = np.arange(128)
    ixw = np.empty((128, 4), dtype=np.int32)
    for blk in range(4):
        ixw[:, blk] = (blk * 128 + pidx) * NGP + A[pidx] + Mlo - 1
    assert ixw.min() >= 0 and ixw.max() + RUNP <= 4 * 128 * NGP

    ah, am, al = _split3(xg * xg)
    bh, bm, bl = _split3(xg)
    xgp = np.stack([ah, am, al, bh, bh, bh, bm, bm, bl]).astype(BF16)

    nc = _get_nc(minus_c, NGP, MT, M2P, RUNP)

    idm = np.eye(128, dtype=BF16)
    in_maps = []
    for i in range(NCORES):
        bsl = slice(i * B_LOC, (i + 1) * B_LOC)
        qc = q[bsl].reshape(B_LOC * P)
        lgc = lg[bsl]
        wq = np.empty((NSETS, 9, 128), dtype=BF16)
        ebias = np.empty((128, NSETS), dtype=np.float32)
        for s in range(NSETS):
            qs = qc[s * 128:(s + 1) * 128]
            wh, wm, wl = _split3(-2.0 * qs)
            one = np.ones(128, dtype=np.float32)
            wq[s] = np.stack([one, one, one, wh, wm, wl, wh, wm, wh]
                             ).astype(BF16)
            ebias[:, s] = minus_c * qs * qs
        in_maps.append({
            "xg": xgp, "wq": wq, "eb": ebias,
            "lt": np.ascontiguousarray(
                lgc.transpose(0, 2, 1).reshape(B_LOC * P, F)
            ).astype(np.float16),
            "oc": OC, "ca": caw, "ix": ixw, "idm": idm,
        })

    kw = {}
    if _trace:
        kw = {"trace": True, "tmpdir": _tmpdir}
    res = run_bass_kernel_spmd(nc, in_maps, core_ids=list(range(NCORES)), **kw)

    out = np.empty((B, V, F), dtype=np.float32)
    for i in range(NCORES):
        oc = res.results[i]["out"].astype(np.float32)   # (2, 128, 512)
        zc = res.results[i]["zout"]                     # (128 f, 4 b)
        for b in range(B_LOC):
            for vh in range(2):
                out[i * B_LOC + b, vh * 128:(vh + 1) * 128, :] \
                    = oc[vh, :, b * 128:(b + 1) * 128] / zc[:, b][None, :]
    if _trace:
        return out, res
    return out


# revision 16
# speedup vs baseline: 2.8825x; 1.0086x over previous
"""Trainium2 Bass kernel for nn_ExtractionLayer — log-grid table algorithm.

metric[b,v,f] = sum_p amp[b,f,p] * G(x_vf - q_bp),  x_vf = fil_f*vol_v,
G = Gaussian of width sig_eff = sigma+0.001, amp = softmax_p(logits).

Instead of 8.4M exps/core (direct method), build the amp-weighted Gaussian
mixture h_{b,f} on a LOG-SPACED x-grid (NG~1.5K points; exps = NG*256 only)
and evaluate metric by two nested cubic (Catmull-Rom) interpolations:
  position(x) = ln(x/x0)/eps = a_f + c_v   (separable in log space!)
  stage alpha: per-(f,b) window gather of T rows (indirect DMA) + fractional
               defrac with per-partition cubic weights (DVE)
  stage gamma: dense matmul with a host-built interp matrix OC[m, v].

Per-core flow (4 b's/core, data-parallel over batch):
  1. elt=exp(lt) (ACT), block-diag Wamp (DVE), Z via PE ones-matmul (host
     divides output by Z).
  2. K[(bp), g] = exp(-c*(x_g - q)^2) via the 9-row bf16-split matmul + ACT,
     g in NGP cols.  (~1.4K*256 exps = ~3us)
  3. T[rfb, g] = sum_p amp*K: PE matmuls, lhsT = Wamp cols, ACCUMULATED over
     both (bp) set-tiles; rfb = b*128+f on partitions. -> bf16 -> DRAM.
  4. gather: ONE indirect DMA pulls per-(f,b) windows T[rfb, A_f+Mlo-1 : +RUN]
     (coef=1 flat indexing, 512 descriptors).
  5. defrac (stage alpha): T2[rfb, m] = sum_r ca_r(alpha_f)*Tw[rfb, m+r].
  6. PE transpose T2 -> [m-part, rfb], then out[v, rfb] = OC^T @ T2t
     accumulated over m-tiles. -> fp16 -> DRAM out.
Host packs: grid splits, Wq/ebias, OC, defrac coeffs, gather indices.
"""

import sys

for _p in ("/opt/trn_rl_repo", "/root/.axon_site/_ro/trn_rl_repo"):
    if _p not in sys.path:
        sys.path.append(_p)

import numpy as np
import ml_dtypes

BF16 = ml_dtypes.bfloat16

B, V, F, P = 32, 256, 128, 64
NCORES = 8
B_LOC = B // NCORES
NSETS = B_LOC // 2
EPS_FRAC = 0.9

_cache: dict = {}


def _build(minus_c, NGP, MT, M2P, RUNP):
    import concourse.tile as tile
    from concourse import bacc, mybir

    fp32 = mybir.dt.float32
    fp16 = mybir.dt.float16
    bf16 = mybir.dt.bfloat16
    int32 = mybir.dt.int32
    AF = mybir.ActivationFunctionType
    OP = mybir.AluOpType
    import concourse.bass as bass

    NGC = NGP // 512            # K/T column chunks
    MTC = MT // 128             # m partition-tiles

    nc = bacc.Bacc("TRN2", target_bir_lowering=False, debug=False,
                   num_devices=NCORES)

    d_xg = nc.dram_tensor("xg", [9, NGP], bf16, kind="ExternalInput")
    d_wq = nc.dram_tensor("wq", [NSETS, 9, 128], bf16, kind="ExternalInput")
    d_eb = nc.dram_tensor("eb", [128, NSETS], fp32, kind="ExternalInput")
    d_lt = nc.dram_tensor("lt", [B_LOC * P, F], fp16, kind="ExternalInput")
    d_oc = nc.dram_tensor("oc", [MT, 256], bf16, kind="ExternalInput")
    d_ca = nc.dram_tensor("ca", [128, 4], fp32, kind="ExternalInput")
    d_ix = nc.dram_tensor("ix", [128, 4], int32, kind="ExternalInput")
    d_id = nc.dram_tensor("idm", [128, 128], bf16, kind="ExternalInput")
    d_t = nc.dram_tensor("tsc", [4 * 128 * NGP], bf16)     # T scratch, flat
    d_out = nc.dram_tensor("out", [2, 128, 512], fp16, kind="ExternalOutput")
    d_z = nc.dram_tensor("zout", [128, 2 * NSETS], fp32, kind="ExternalOutput")

    with tile.TileContext(nc) as tc:
        with (
            tc.tile_pool(name="const", bufs=1) as cp,
            tc.tile_pool(name="ps", bufs=2, space=bass.MemorySpace.PSUM) as ps,
        ):
            # ---- loads ----
            xg_sb = cp.tile([9, NGP], bf16, tag="xg")
            Wq = [cp.tile([9, 128], bf16, tag=f"Wq{s}", name=f"Wq{s}")
                  for s in range(NSETS)]
            eb = cp.tile([128, NSETS], fp32, tag="eb")
            lt_sb = [cp.tile([128, F], fp16, tag=f"lt{s}", name=f"lt{s}")
                     for s in range(NSETS)]
            oc_sb = [cp.tile([128, 256], bf16, tag=f"oc{t}", name=f"oc{t}")
                     for t in range(MTC)]
            ca = cp.tile([128, 4], fp32, tag="ca")
            ix = cp.tile([128, 4], int32, tag="ix")
            ident = cp.tile([128, 128], bf16, tag="ident")

            nc.sync.dma_start(xg_sb[:, 0:512], d_xg.ap()[:, 0:512])
            nc.sync.dma_start(xg_sb[:, 512:NGP], d_xg.ap()[:, 512:NGP])
            for s in range(NSETS):
                nc.gpsimd.dma_start(Wq[s][:, :], d_wq.ap()[s])
            nc.gpsimd.dma_start(eb[:, :], d_eb.ap())
            nc.gpsimd.dma_start(ca[:, :], d_ca.ap())
            nc.gpsimd.dma_start(ix[:, :], d_ix.ap())
            for s in range(NSETS):
                nc.scalar.dma_start(lt_sb[s][:, :],
                                    d_lt.ap()[s * 128:(s + 1) * 128, :])
            nc.sync.dma_start(ident[:, :], d_id.ap())
            for t in range(MTC):
                nc.sync.dma_start(oc_sb[t][:, :],
                                  d_oc.ap()[t * 128:(t + 1) * 128, :])

            # ---- softmax numerators + Z ----
            elt = []
            for s in range(NSETS):
                e = cp.tile([128, F], fp16, tag=f"elt{s}", name=f"elt{s}")
                nc.scalar.activation(e[:, :], lt_sb[s][:, :], AF.Exp)
                elt.append(e)
            Wamp = []
            for s in range(NSETS):
                w = cp.tile([128, 512], fp16, tag=f"Wamp{s}", name=f"Wamp{s}")
                nc.vector.memset(w[:, :], 0.0)
                for h in range(2):
                    b = 2 * s + h
                    nc.vector.tensor_copy(
                        w[h * 64:(h + 1) * 64, b * 128:(b + 1) * 128],
                        elt[s][h * 64:(h + 1) * 64, :])
                Wamp.append(w)
            ones2 = cp.tile([128, 2], fp16, tag="ones2")
            nc.vector.memset(ones2[:, :], 0.0)
            nc.vector.memset(ones2[0:64, 0:1], 1.0)
            nc.vector.memset(ones2[64:128, 1:2], 1.0)
            Zc = cp.tile([128, 2 * NSETS], fp32, tag="Zc")
            for s in range(NSETS):
                zp = ps.tile([128, 512], fp32, tag="R", name=f"zp{s}")
                nc.tensor.matmul(zp[:, 0:2], elt[s][:, :], ones2[:, :],
                                 start=True, stop=True)
                nc.vector.tensor_copy(Zc[:, 2 * s:2 * s + 2], zp[:, 0:2])

            # ---- K (all chunks), then per-b-block pipelined T build ->
            # DMA -> gather -> defrac -> transpose ----
            K_sb = [cp.tile([128, NGP], fp16, tag=f"K{s}", name=f"K{s}")
                    for s in range(NSETS)]
            T_sb = [cp.tile([128, NGP], bf16, tag=f"Tsb{t}", name=f"Tsb{t}")
                    for t in range(4)]
            Tw = cp.tile([128, 4 * RUNP], bf16, tag="Tw")
            T2 = cp.tile([128, 4 * MT], bf16, tag="T2")
            tmp = cp.tile([128, M2P], bf16, tag="tmp")
            T2t = [cp.tile([128, 512], bf16, tag=f"T2t{t}", name=f"T2t{t}")
                   for t in range(MTC)]
            nc.vector.memset(T2[:, :], 0.0)
            teng = [nc.sync, nc.scalar]
            ti = 0

            for c in range(NGC):
                for s in range(NSETS):
                    psK = ps.tile([128, 512], fp32, tag="K",
                                  name=f"psK{c}_{s}")
                    nc.tensor.matmul(psK[:, :], Wq[s][:, :],
                                     xg_sb[:, c * 512:(c + 1) * 512],
                                     start=True, stop=True)
                    nc.scalar.activation(K_sb[s][:, c * 512:(c + 1) * 512],
                                         psK[:, :], AF.Exp,
                                         scale=float(minus_c),
                                         bias=eb[:, s:s + 1])

            # all T matmuls + ACT drains + DMAs, then gathers chase them
            for t in range(4):
                for c in range(NGC):
                    psT = ps.tile([128, 512], fp32, tag="T", name=f"psT{c}_{t}")
                    for s in range(NSETS):
                        nc.tensor.matmul(
                            psT[:, :], Wamp[s][:, t * 128:(t + 1) * 128],
                            K_sb[s][:, c * 512:(c + 1) * 512],
                            start=(s == 0), stop=(s == NSETS - 1))
                    if ti % 2 == 0:
                        nc.vector.tensor_copy(
                            T_sb[t][:, c * 512:(c + 1) * 512], psT[:, :])
                    else:
                        nc.scalar.activation(
                            T_sb[t][:, c * 512:(c + 1) * 512], psT[:, :],
                            AF.Copy)
                    teng[ti % 2].dma_start(
                        bass.AP(tensor=d_t, offset=t * 128 * NGP + c * 512,
                                ap=[[NGP, 128], [1, 512]]),
                        T_sb[t][:, c * 512:(c + 1) * 512])
                    ti += 1
                # gather this block's per-f windows (one idx per out row)
                nc.gpsimd.indirect_dma_start(
                    out=Tw[:, t * RUNP:(t + 1) * RUNP],
                    out_offset=None,
                    in_=bass.AP(tensor=d_t, offset=0,
                                ap=[[1, t * 128 * NGP + 128 * NGP], [1, 1]]),
                    in_offset=bass.IndirectOffsetOnAxis(
                        ap=ix[:, t:t + 1], axis=0),
                )

            # defrac (DVE) -> transpose (PE) -> per-block final matmuls
            psO = [ps.tile([128, 512], fp32, tag="R", name=f"psO{vh}")
                   for vh in range(2)]
            tmps = [cp.tile([128, M2P], bf16, tag=f"tm{r}", name=f"tm{r}")
                    for r in range(3)]
            for t in range(4):
                o = t * RUNP
                m = t * MT
                nc.vector.tensor_scalar(T2[:, m:m + M2P], Tw[:, o:o + M2P],
                                        ca[:, 0:1], None, op0=OP.mult)
                nc.scalar.activation(tmps[0][:, :], Tw[:, o + 1:o + 1 + M2P],
                                     AF.Copy, scale=ca[:, 1:2])
                nc.scalar.activation(tmps[1][:, :], Tw[:, o + 2:o + 2 + M2P],
                                     AF.Copy, scale=ca[:, 2:3])
                nc.vector.tensor_scalar(tmps[2][:, :], Tw[:, o + 3:o + 3 + M2P],
                                        ca[:, 3:4], None, op0=OP.mult)
                for r in range(1, 4):
                    nc.vector.tensor_tensor(T2[:, m:m + M2P], T2[:, m:m + M2P],
                                            tmps[r - 1][:, :], OP.add)
                for mt in range(MTC):
                    psR = ps.tile([128, 128], bf16, tag="Rb",
                                  name=f"psR{t}_{mt}")
                    nc.tensor.transpose(
                        psR[:, :],
                        T2[:, t * MT + mt * 128:t * MT + (mt + 1) * 128],
                        ident[:, :])
                    nc.vector.tensor_copy(
                        T2t[mt][:, t * 128:(t + 1) * 128], psR[:, :])
                for vh in range(2):
                    for mt in range(MTC):
                        nc.tensor.matmul(
                            psO[vh][:, t * 128:(t + 1) * 128],
                            oc_sb[mt][:, vh * 128:(vh + 1) * 128],
                            T2t[mt][:, t * 128:(t + 1) * 128],
                            start=(mt == 0), stop=(mt == MTC - 1))

            # ---- drain outputs ----
            ob0 = cp.tile([128, 512], fp16, tag="ob0")
            nc.vector.tensor_copy(ob0[:, :], psO[0][:, :])
            nc.sync.dma_start(d_out.ap()[0], ob0[:, :])
            ob1 = cp.tile([128, 512], fp16, tag="ob1")
            nc.scalar.activation(ob1[:, :], psO[1][:, :], AF.Copy)
            nc.scalar.dma_start(d_out.ap()[1], ob1[:, :])
            nc.sync.dma_start(d_z.ap(), Zc[:, :])

    nc.compile()
    return nc


def _get_nc(minus_c, NGP, MT, M2P, RUNP):
    key = (round(float(minus_c), 4), NGP, MT, M2P, RUNP)
    if key not in _cache:
        _cache[key] = _build(minus_c, NGP, MT, M2P, RUNP)
    return _cache[key]


def _split3(v):
    h = v.astype(BF16)
    r1 = v - h.astype(np.float32)
    m = r1.astype(BF16)
    r2 = r1 - m.astype(np.float32)
    return h.astype(np.float32), m.astype(np.float32), r2.astype(BF16)


def _cr(t):
    t2 = t * t
    t3 = t2 * t
    return np.stack([-0.5 * t3 + t2 - 0.5 * t, 1.5 * t3 - 2.5 * t2 + 1,
                     -1.5 * t3 + 2 * t2 + 0.5 * t, 0.5 * t3 - 0.5 * t2])


def kernel(q2_obs_scaled, amplitude_logits, volumes, filters, sigma,
           _trace=False, _tmpdir=None):
    from concourse.bass_utils import run_bass_kernel_spmd

    sig_eff = float(np.asarray(sigma).reshape(())) + 0.001
    minus_c = -0.5 / sig_eff ** 2

    q = np.ascontiguousarray(np.asarray(q2_obs_scaled, dtype=np.float32))
    lg = np.asarray(amplitude_logits, dtype=np.float32).reshape(B, F, P)
    vol = np.ascontiguousarray(np.asarray(volumes, dtype=np.float32).reshape(V))
    fil = np.ascontiguousarray(np.asarray(filters, dtype=np.float32).reshape(F))

    # ---- grid + interpolation geometry (host) ----
    x = fil[:, None] * vol[None, :]
    xmax = float(x.max())
    eps = EPS_FRAC * sig_eff / xmax
    x0 = float(x.min()) * 0.999 * np.exp(-8 * eps)
    NG = int(np.ceil(np.log(xmax / x0) / eps)) + 10
    NGP = ((NG + 511) // 512) * 512
    xg = (x0 * np.exp(eps * np.arange(NGP))).astype(np.float32)
    af = np.log(fil / x0) / eps
    cv = np.log(vol) / eps
    A = np.floor(af).astype(np.int64)
    alpha = (af - A).astype(np.float32)
    C = np.floor(cv).astype(np.int64)
    gamma = (cv - C).astype(np.float32)
    Mlo = int(C.min()) - 2
    M2 = int(C.max()) + 3 - Mlo
    M2P = ((M2 + 3) // 4) * 4
    RUNP = M2P + 4
    MT = ((M2P + 127) // 128) * 128

    wC = _cr(gamma)                                     # [4, V]
    OC = np.zeros((MT, 256), dtype=np.float32)
    for j in range(4):
        OC[(C - Mlo) + j - 1, np.arange(256)] += wC[j]
    OC = OC.astype(BF16)
    caw = _cr(alpha).T.astype(np.float32)               # [128, 4]
    # gather start: flat index of T[rfb, A_f + Mlo - 1]
    pidx = np.arange(128)
    ixw = np.empty((128, 4), dtype=np.int32)
    for blk in range(4):
        ixw[:, blk] = (blk * 128 + pidx) * NGP + A[pidx] + Mlo - 1
    assert ixw.min() >= 0 and ixw.max() + RUNP <= 4 * 128 * NGP

    ah, am, al = _split3(xg * xg)
    bh, bm, bl = _split3(xg)
    xgp = np.stack([ah, am, al, bh, bh, bh, bm, bm, bl]).astype(BF16)

    nc = _get_nc(minus_c, NGP, MT, M2P, RUNP)

    idm = np.eye(128, dtype=BF16)
    in_maps = []
    for i in range(NCORES):
        bsl = slice(i * B_LOC, (i + 1) * B_LOC)
        qc = q[bsl].reshape(B_LOC * P)
        lgc = lg[bsl]
        wq = np.empty((NSETS, 9, 128), dtype=BF16)
        ebias = np.empty((128, NSETS), dtype=np.float32)
        for s in range(NSETS):
            qs = qc[s * 128:(s + 1) * 128]
            wh, wm, wl = _split3(-2.0 * qs)
            one = np.ones(128, dtype=np.float32)
            wq[s] = np.stack([one, one, one, wh, wm, wl, wh, wm, wh]
                             ).astype(BF16)
            ebias[:, s] = minus_c * qs * qs
        in_maps.append({
            "xg": xgp, "wq": wq, "eb": ebias,
            "lt": np.ascontiguousarray(
                lgc.transpose(0, 2, 1).reshape(B_LOC * P, F)
            ).astype(np.float16),
            "oc": OC, "ca": caw, "ix": ixw, "idm": idm,
        })

    kw = {}
    if _trace:
        kw = {"trace": True, "tmpdir": _tmpdir}
    res = run_bass_kernel_spmd(nc, in_maps, core_ids=list(range(NCORES)), **kw)

    out = np.empty((B, V, F), dtype=np.float32)
    for i in range(NCORES):
        oc = res.results[i]["out"].astype(np.float32)   # (2, 128, 512)
        zc = res.results[i]["zout"]                     # (128 f, 4 b)
        for b in range(B_LOC):
            for vh in range(2):
                out[i * B_LOC + b, vh * 128:(vh + 1) * 128, :] \
                    = oc[vh, :, b * 128:(b + 1) * 128] / zc[:, b][None, :]
    if _trace:
        return out, res
    return out
